# revision 1
# baseline (speedup 1.0000x reference)
"""DiagonalLSTM Bass/Tile kernel for TRN2 (per-core shard: B=4 images).

Design "plain-p" (v2):
  - INP [66, 32768] bf16: padded skewed-readable input. col = 8192*b + 128*p + w
    holds inputs[b,c,p,w]; the 64-col gap after each row is zero, so the
    diagonal read  col = 8192*b + 127*p + t  yields skewed x (zero outside
    the band) for ANY p with no per-step copies. Partitions 64,65 = 1.0
    (bias rows for the K=66 i2s matmul: row 64 = b_i2s, row 65 = b_s2s).
  - H [65, 260] bf16: h-state, col = 65*b + 1 + p; col 65*b = 0 (so a
    -1-offset view gives h[p-1] with a zero boundary); part 64 = 1.0 (bias
    row for the K=65 upsample matmul).
  - P [128, 512] f32 PSUM: gate preacts, col = 256*w' + 64*b + p' (w' =
    channel block). All biases ride matmul ones-rows -> bias-free sigmoid.
  - Flat-split quirk: preact (row p' = 16q + r, chan m = 64s + k) feeds
    (gate q, state p2 = 4r + 2w' + u, chan k) with u = s%2 (partition half),
    w' = s//2. G [128,512] col = 128q + 32b + kap2 (kap2 = 2r + w' = p2>>1);
    sigmoid is 2 scatter instrs (one per w'). State tiles C2N/CP/TH/T1/T2
    are [128 parts = 64u+k, 128 cols = 32b + kap2] so all gate math is
    2-free-dim views; gate-q views of G are [[32,4],[1,32]] at offset 128q.
  - c2c: CC [128,128] bf16 copy of C2N; 4 matmuls (parity taps, lhsT
    duplicated at partition 64). c2c bias enters via scalar_tensor_tensor.
  - Upsample: K=65 matmul (bias via H ones row) -> U PSUM -> scalar-engine
    copy into OUT [128,16384] bf16 band view; final 4 contiguous DMAs.
"""
import os
from contextlib import ExitStack

import numpy as np

import concourse.bass as bass
import concourse.tile as tile
from concourse import bacc, mybir

F32 = mybir.dt.float32
BF = mybir.dt.bfloat16
AF = mybir.ActivationFunctionType
ALU = mybir.AluOpType

B = 4          # images per core
H = 64         # rows
W = 64         # cols
C = 64         # input channels
HID = 64       # hidden
NW = H + W - 1 # 127 diagonal steps

SB = 8192      # INP col stride per image


def v(ap, off, dims):
    """View: keep ap's partition dim, replace free dims, add offset (elems)."""
    return bass.AP(ap.tensor, ap.offset + off, [list(ap.ap[0])] + [list(d) for d in dims])


def dv(ap, off, dims):
    """Fully-custom view (DRAM side of DMAs)."""
    return bass.AP(ap.tensor, off, [list(d) for d in dims])


def band(t):
    return max(0, t - (W - 1)), min(H - 1, t)


def build_kernel(ctx, tc, outs, ins):
    nc = tc.nc
    x_d = ins["inputs"]
    out_d = outs["out"]

    const = ctx.enter_context(tc.tile_pool(name="const", bufs=1))
    big = ctx.enter_context(tc.tile_pool(name="big", bufs=1))
    st = ctx.enter_context(tc.tile_pool(name="st", bufs=2))
    tmp = ctx.enter_context(tc.tile_pool(name="tmp", bufs=2))
    ps = ctx.enter_context(tc.tile_pool(name="ps", bufs=2, space="PSUM"))

    # ---------------- weights / biases (one-time prep) ----------------
    w_i2s = ins["w_i2s"]   # [256, 64] dram f32
    b_i2s = ins["b_i2s"]   # [256]
    w_s2s = ins["w_s2s"]   # [256, 64, 2]
    b_s2s = ins["b_s2s"]   # [256]
    w_c2c = ins["w_c2c"]   # [64, 64, 2]
    b_c2c = ins["b_c2c"]   # [64]
    w_up = ins["w_up"]     # [128, 64]
    b_up = ins["b_up"]     # [128]

    # f32 staging
    LAf = [const.tile([66, 128], F32, tag=f"LAf{w}", name=f"LAf{w}") for w in range(2)]
    LS1f = [const.tile([64, 128], F32, tag=f"LS1f{w}", name=f"LS1f{w}") for w in range(2)]
    LS0f = [const.tile([64, 128], F32, tag=f"LS0f{w}", name=f"LS0f{w}") for w in range(2)]
    LC1f = const.tile([64, 64], F32, tag="LC1f")
    LC0f = const.tile([64, 64], F32, tag="LC0f")
    LUPf = const.tile([65, 128], F32, tag="LUPf")
    # bf16 lhsT tiles
    LA = [const.tile([66, 128], BF, tag=f"LA{w}", name=f"LA{w}") for w in range(2)]
    LS1 = [const.tile([64, 128], BF, tag=f"LS1{w}", name=f"LS1{w}") for w in range(2)]
    LS0 = [const.tile([64, 128], BF, tag=f"LS0{w}", name=f"LS0{w}") for w in range(2)]
    LC1 = const.tile([64, 64], BF, tag="LC1")
    LC0 = const.tile([64, 64], BF, tag="LC0")
    LUP = const.tile([65, 128], BF, tag="LUP")
    bc2c = const.tile([128, 1], F32, tag="bc2c")

    for w in range(2):
        # LA rows 0:64 = Wi2s[128w+m, c] -> lhsT[c, m]; row 64 = b_i2s; row 65 = b_s2s
        nc.sync.dma_start(out=LAf[w][0:64, :], in_=dv(w_i2s, 128 * w * 64, [[1, 64], [64, 128]]))
        nc.sync.dma_start(out=LAf[w][64:65, :], in_=dv(b_i2s, 128 * w, [[1, 1], [1, 128]]))
        nc.sync.dma_start(out=LAf[w][65:66, :], in_=dv(b_s2s, 128 * w, [[1, 1], [1, 128]]))
        # LS1/LS0: w_s2s[(128w+m), h, tap] -> lhsT[h, m]
        nc.sync.dma_start(out=LS1f[w][:, :], in_=dv(w_s2s, 128 * w * 128 + 1, [[2, 64], [128, 128]]))
        nc.sync.dma_start(out=LS0f[w][:, :], in_=dv(w_s2s, 128 * w * 128 + 0, [[2, 64], [128, 128]]))
    nc.sync.dma_start(out=LC1f[:, :], in_=dv(w_c2c, 1, [[2, 64], [128, 64]]))
    nc.sync.dma_start(out=LC0f[:, :], in_=dv(w_c2c, 0, [[2, 64], [128, 64]]))
    # upsample: rows 0:64 = w_up.T; row 64 = b_up
    nc.sync.dma_start(out=LUPf[0:64, :], in_=dv(w_up, 0, [[1, 64], [64, 128]]))
    nc.sync.dma_start(out=LUPf[64:65, :], in_=dv(b_up, 0, [[1, 1], [1, 128]]))
    # c2c bias [128,1] (both halves)
    nc.sync.dma_start(out=bc2c[0:64, :], in_=dv(b_c2c, 0, [[1, 64], [1, 1]]))
    nc.sync.dma_start(out=bc2c[64:128, :], in_=dv(b_c2c, 0, [[1, 64], [1, 1]]))

    for bf_t, f_t in ((LA[0], LAf[0]), (LA[1], LAf[1]), (LS1[0], LS1f[0]),
                      (LS1[1], LS1f[1]), (LS0[0], LS0f[0]), (LS0[1], LS0f[1]),
                      (LC1, LC1f), (LC0, LC0f), (LUP, LUPf)):
        nc.vector.tensor_copy(bf_t[:, :], f_t[:, :])

    # ---------------- input (padded) + output tiles ----------------
    # col = 8192*b + 64*t + p  (entry (p,t) of the skewed image, in-band
    # entries only; everything else stays zero). Step t's x column is the
    # CONTIGUOUS 64-col run at offset 64*t.
    INP = big.tile([66, 4 * SB], BF, tag="INP")
    for j in range(8):
        ch = 4 * SB // 8
        nc.sync.dma_start(
            out=v(INP[0:66, :], j * ch, [[1, ch]]),
            in_=dv(x_d, j * ch, [[4 * SB, 66], [1, ch]]),
        )

    OUT = big.tile([128, B * H * W], BF, tag="OUT")
    OUT_ap = OUT[:, :]

    # ---------------- state tiles ----------------
    H0 = st.tile([65, 4 * 65], BF, tag="Ht", name="H0", bufs=2)
    H1 = st.tile([65, 4 * 65], BF, tag="Ht", name="H1", bufs=2)
    for Ht in (H0, H1):
        nc.vector.memset(Ht[0:64, :], 0.0)
        nc.gpsimd.memset(Ht[64:65, :], 1.0)
    CCe0 = st.tile([64, 128], BF, tag="CCe", name="CCe0", bufs=2)
    nc.vector.memset(CCe0[:, :], 0.0)
    CCo0 = st.tile([64, 128], BF, tag="CCo", name="CCo0", bufs=2)
    nc.vector.memset(CCo0[:, :], 0.0)

    Hprev = H0
    CCeprev = CCe0
    CCoprev = CCo0


    BK = [[32, 4], [1, 32]]   # (b, kap2) on state tiles / G gate views

    # ---------------- the recurrence ----------------
    nsteps = int(os.environ.get("NWN", NW))
    pending_up = None  # (t, Hn) of the previous step

    def emit_up(pt, pH):
        U = ps.tile([128, 256], F32, tag="U")
        nc.tensor.matmul(
            v(U[:, :], 0, [[64, 4], [1, 64]]),
            LUP[0:65, :],
            v(pH[0:65, :], 1, [[65, 4], [1, 64]]),
            start=True, stop=True,
        )
        lo, hi = band(pt)
        n = hi - lo + 1
        nc.vector.tensor_copy(
            v(OUT_ap, 63 * lo + pt, [[4096, 4], [63, n]]),
            v(U[:, :], lo, [[64, 4], [1, n]]),
        )
    F_NOOUT = "DBG_NOOUT" in os.environ    # skip upsample + OUT copy
    F_NOI2S = "DBG_NOI2S" in os.environ    # skip i2s matmuls
    F_DSIG = "DBG_DSIG" in os.environ      # dense sigmoid (no scatter)
    F_NOC2C = "DBG_NOC2C" in os.environ    # skip c2c matmuls (+dummy T2)
    F_NOGP = "DBG_NOGP" in os.environ      # gpsimd ops on vector instead
    for t in range(nsteps):
        P = ps.tile([128, 512], F32, tag="P")
        CP = ps.tile([128, 128], F32, tag="CP")
        Pap = P[:, :]

        # --- PE: i2s (+ gate biases via ones rows), x read straight from INP
        if not F_NOI2S:
            for w in range(2):
                # start=True only on the FIRST matmul touching the bank:
                # first_mm clears has_written for the WHOLE bank.
                nc.tensor.matmul(
                    v(Pap, 256 * w, [[64, 4], [1, 64]]),
                    LA[w][0:66, :],
                    v(INP[0:66, :], 64 * t, [[SB, 4], [1, 64]]),
                    start=(w == 0), stop=False, skip_group_check=True,
                )
        # --- PE: h-dependent gate matmuls
        for w in range(2):
            nc.tensor.matmul(
                v(Pap, 256 * w, [[64, 4], [1, 64]]),
                LS1[w][:, :],
                v(Hprev[0:64, :], 1, [[65, 4], [1, 64]]),
                start=F_NOI2S, stop=False, skip_group_check=True,
            )
            nc.tensor.matmul(
                v(Pap, 256 * w, [[64, 4], [1, 64]]),
                LS0[w][:, :],
                v(Hprev[0:64, :], 0, [[65, 4], [1, 64]]),
                start=False, stop=True, skip_group_check=True,
            )
        # --- previous step's upsample + OUT write (runs in this sigmoid window)
        if pending_up is not None and not F_NOOUT:
            emit_up(*pending_up)
        # --- PE: c2c (reads CCprev bf16); runs in the sigmoid window
        CPlo = CP[0:64, :]
        CPhi = CP[64:128, :]
        if not F_NOC2C:
            # u=0 direct: c[p2=2kap2] <- Wc1 @ c-even
            nc.tensor.matmul(CPlo, LC1[:, :], CCeprev[:, :],
                             start=True, stop=False, skip_group_check=True)
            # u=0 tap: c[2kap2 - 1] = c-odd at kap2-1 (kap2=0 -> zero, skipped)
            nc.tensor.matmul(
                v(CPlo, 1, [[32, 4], [1, 31]]),
                LC0[:, :],
                v(CCoprev[:, :], 0, [[32, 4], [1, 31]]),
                start=False, stop=True, skip_group_check=True,
            )
            # u=1 direct: Wc1 @ c-odd. start=True: the has_written clear is
            # per (partition, bank); this matmul writes partitions 64:128
            # which the u=0 matmul (parts 0:64) did not clear.
            nc.tensor.matmul(CPhi, LC1[:, :], CCoprev[:, :],
                             start=True, stop=False, skip_group_check=True)
            # u=1 tap: c[2kap2] = c-even same kap2
            nc.tensor.matmul(CPhi, LC0[:, :], CCeprev[:, :],
                             start=False, stop=True, skip_group_check=True)

        # --- ACT: sigmoid (2 scatter instrs, one per channel block w')
        G = tmp.tile([128, 512], F32, tag="G")
        Gap = G[:, :]
        if F_DSIG:
            nc.scalar.activation(Gap, Pap, AF.Sigmoid)
        else:
            for w in range(2):
                nc.scalar.activation(
                    v(Gap, w, [[32, 4], [128, 4], [2, 16]]),
                    v(Pap, 256 * w, [[64, 4], [16, 4], [1, 16]]),
                    AF.Sigmoid,
                )

        # --- GPSIMD: T1 = ig * gg
        T1 = tmp.tile([128, 128], F32, tag="T1")
        eng_t1 = nc.vector if F_NOGP else nc.gpsimd
        eng_t1.tensor_mul(
            v(T1[:, :], 0, BK), v(Gap, 0, BK), v(Gap, 128, BK),
        )
        # --- DVE: T2 = (CP + b_c2c) * fg
        T2 = tmp.tile([128, 128], F32, tag="T2")
        if F_NOC2C:
            nc.vector.tensor_mul(v(T2[:, :], 0, BK), v(Gap, 256, BK), v(Gap, 256, BK))
        else:
            nc.vector.scalar_tensor_tensor(
                out=v(T2[:, :], 0, BK),
                in0=v(CP[:, :], 0, BK),
                scalar=bc2c[:, 0:1],
                in1=v(Gap, 256, BK),
                op0=ALU.add, op1=ALU.mult,
            )
        # --- DVE: c = T1 + T2
        C2N = ps.tile([128, 128], F32, tag="C2N")
        nc.vector.tensor_add(C2N[:, :], T1[:, :], T2[:, :])
        # --- ACT: tanh
        TH = tmp.tile([128, 128], BF, tag="TH")
        nc.scalar.activation(TH[:, :], C2N[:, :], AF.Tanh)
        # --- h = og * tanh(c): u=0 on DVE, u=1 on GPSIMD (parallel)
        Hn = st.tile([65, 4 * 65], BF, tag="Ht", name="Hn", bufs=2)
        nc.vector.tensor_mul(
            v(Hn[0:64, :], 1, [[65, 4], [2, 32]]),
            v(G[0:64, :], 384, BK),
            v(TH[0:64, :], 0, BK),
        )
        eng_h1 = nc.vector if F_NOGP else nc.gpsimd
        eng_h1.tensor_mul(
            v(Hn[0:64, :], 2, [[65, 4], [2, 32]]),
            v(G[64:128, :], 384, BK),
            v(TH[64:128, :], 0, BK),
        )
        # --- GPSIMD: CC = bf16(c) for next step's c2c
        CCen = st.tile([64, 128], BF, tag="CCe", name="CCen", bufs=2)
        CCon = st.tile([64, 128], BF, tag="CCo", name="CCon", bufs=2)
        nc.vector.tensor_copy(CCen[:, :], C2N[0:64, :])
        nc.vector.tensor_copy(CCon[:, :], C2N[64:128, :])

        if "DBG_DUMP" in os.environ and t == int(os.environ.get("DBG_DUMP_T", 0)):
            nc.sync.dma_start(out=dv(outs["gdump"], 0, [[512, 128], [1, 512]]), in_=G[:, :])
            nc.sync.dma_start(out=dv(outs["la0dump"], 0, [[128, 66], [1, 128]]), in_=LA[0][:, :])
            nc.sync.dma_start(out=dv(outs["la1dump"], 0, [[128, 66], [1, 128]]), in_=LA[1][:, :])
            PD = tmp.tile([128, 512], F32, tag="PD")
            nc.vector.tensor_copy(PD[:, :], Pap)
            nc.sync.dma_start(out=dv(outs["pdump"], 0, [[512, 128], [1, 512]]), in_=PD[:, :])
            nc.sync.dma_start(out=dv(outs["hdump"], 0, [[260, 65], [1, 260]]), in_=Hn[:, :])
            nc.sync.dma_start(out=dv(outs["cdump"], 0, [[128, 128], [1, 128]]), in_=C2N[:, :])
            nc.sync.dma_start(out=dv(outs["thdump"], 0, [[128, 128], [1, 128]]), in_=TH[:, :])

        pending_up = (t, Hn)
        Hprev = Hn
        CCeprev = CCen
        CCoprev = CCon

    if pending_up is not None and not F_NOOUT:
        emit_up(*pending_up)

    # ---------------- output store ----------------
    if F_NOOUT or nsteps == 0:
        nc.vector.memset(OUT_ap, 0.0)
    # 16-way split spreads the store across all DMA queues
    for b in range(B):
        for j in range(4):
            nc.sync.dma_start(
                out=dv(out_d, b * 128 * H * W + j * 1024, [[4096, 128], [1, 1024]]),
                in_=v(OUT_ap, b * H * W + j * 1024, [[1, 1024]]),
            )


def build_nc():
    nc = bacc.Bacc("TRN2", target_bir_lowering=False, debug=False)
    ins = {
        "inputs": nc.dram_tensor("inputs", [66, 4 * SB], BF, kind="ExternalInput").ap(),
        "w_i2s": nc.dram_tensor("w_i2s", [4 * HID, C], F32, kind="ExternalInput").ap(),
        "b_i2s": nc.dram_tensor("b_i2s", [4 * HID], F32, kind="ExternalInput").ap(),
        "w_s2s": nc.dram_tensor("w_s2s", [4 * HID, HID, 2], F32, kind="ExternalInput").ap(),
        "b_s2s": nc.dram_tensor("b_s2s", [4 * HID], F32, kind="ExternalInput").ap(),
        "w_c2c": nc.dram_tensor("w_c2c", [HID, HID, 2], F32, kind="ExternalInput").ap(),
        "b_c2c": nc.dram_tensor("b_c2c", [HID], F32, kind="ExternalInput").ap(),
        "w_up": nc.dram_tensor("w_up", [2 * HID, HID], F32, kind="ExternalInput").ap(),
        "b_up": nc.dram_tensor("b_up", [2 * HID], F32, kind="ExternalInput").ap(),
    }
    outs = {"out": nc.dram_tensor("out", [B, 2 * HID, H, W], BF, kind="ExternalOutput").ap()}
    if "DBG_DUMP" in os.environ:
        outs["gdump"] = nc.dram_tensor("gdump", [128, 512], F32, kind="ExternalOutput").ap()
        outs["pdump"] = nc.dram_tensor("pdump", [128, 512], F32, kind="ExternalOutput").ap()
        outs["la0dump"] = nc.dram_tensor("la0dump", [66, 128], BF, kind="ExternalOutput").ap()
        outs["la1dump"] = nc.dram_tensor("la1dump", [66, 128], BF, kind="ExternalOutput").ap()
        outs["hdump"] = nc.dram_tensor("hdump", [65, 260], BF, kind="ExternalOutput").ap()
        outs["cdump"] = nc.dram_tensor("cdump", [128, 128], F32, kind="ExternalOutput").ap()
        outs["thdump"] = nc.dram_tensor("thdump", [128, 128], BF, kind="ExternalOutput").ap()
    with tile.TileContext(nc) as tc:
        with ExitStack() as ctx:
            build_kernel(ctx, tc, outs, ins)
    nc.compile()
    return nc


# ---------------------------------------------------------------------------
# Harness entry point: full inputs -> shard over 8 cores -> full output.
# ---------------------------------------------------------------------------
import ml_dtypes
from concourse.bass_utils import run_bass_kernel_spmd

N_CORES = 8
TRACE = False
LAST_EXEC_NS = None
_NC = None


def _get_nc():
    global _NC
    if _NC is None:
        _NC = build_nc()
    return _NC


def _skew_pack(xs):
    """[B,C,H,W] bf16 -> [66, 4*SB] bf16: col = SB*b + 64*(p+w) + p,
    partitions 64,65 = 1.0 (matmul bias ones-rows)."""
    arr = np.zeros((66, 4 * SB), ml_dtypes.bfloat16)
    arr[64:66, :] = 1.0
    p_i, w_i = np.meshgrid(np.arange(H), np.arange(W), indexing="ij")
    cols = 64 * (p_i + w_i) + p_i          # [H, W]
    for b in range(B):
        arr[0:64, SB * b + cols.ravel()] = xs[b].reshape(C, H * W)
    return arr


def kernel(**inputs):
    global LAST_EXEC_NS
    nc = _get_nc()
    full = {k: np.ascontiguousarray(np.asarray(val, np.float32))
            for k, val in inputs.items()}
    xs = full["inputs"].astype(ml_dtypes.bfloat16)
    in_maps = []
    for i in range(N_CORES):
        m = dict(full)
        m["inputs"] = np.ascontiguousarray(_skew_pack(xs[B * i:B * (i + 1)]))
        in_maps.append(m)
    res = run_bass_kernel_spmd(nc, in_maps, list(range(N_CORES)), trace=TRACE)
    LAST_EXEC_NS = res.exec_time_ns
    return np.concatenate(
        [res.results[i]["out"].astype(np.float32) for i in range(N_CORES)], axis=0)

